# revision 21
# baseline (speedup 1.0000x reference)
"""BertSelfAttention (B=2, S=2048, H=1024, 16 heads x 64) on 8 TRN2 NeuronCores.

Sharding: data parallel on batch (4 cores per batch) x tensor parallel on
heads (4 heads per core). No cross-core comms; each core computes
out[b, :, 256*g:256*(g+1)] for its head group g.

v6: single unified attention loop over both head pairs with lead-2 score
emission (score PSUM pool bufs=3) so the ACT exp stream free-runs; all
projection PSUM shares the score pool's slots (tag-sharing) and the output
transposes reuse the two ctx banks, staying inside 8 PSUM banks. Startup:
weights/consts load on the scalar HWDGE queue while the serial X-bar
transposes (staircase of 8x [256,128] per 256-row block) start at t=0 on
the sync queue; a PE warmup chain ramps the p-state during the DMA wait.
Projections are emitted as 256-col filler groups spread through the loop
with deadline-based placement. Output DMA per (pair, qb) during the stream.

Per-core pipeline:
  A) hiddenT staircase: 64 X-bar transposes [256,128] -> hidTb[b][128,8,256]
  B) kT/qT [128(d of pair), 2048(s)] bf16 (1/8 scale + bias folded),
     V [128(s), 4heads, 65] bf16 with ones column (denominator for free)
  C) per (pair, qb=512, k): scoresT h0|h1 packed -> one exp [128,1024]
     (mask bias) -> bf16 et, ctxT[65, 512] += v_ext.T @ et per head
  D) PE-transpose ctxT (bf16) -> [q, 65], DVE reciprocal + scale, DMA out
"""

import ml_dtypes
import numpy as np

import concourse.bass as bass
import concourse.tile as tile
from concourse import bacc, mybir
from concourse.bass_utils import run_bass_kernel_spmd
from concourse.masks import make_identity

F32 = mybir.dt.float32
BF16 = mybir.dt.bfloat16
EXP = mybir.ActivationFunctionType.Exp

B, S, H = 2, 2048, 1024
NH, HD = 16, 64
NCORES = 8
HPC = 4  # heads per core
DPC = HPC * HD  # 256 output dims per core
SC = S // 128  # 16 s/k chunks
JC = H // 128  # 8 contraction chunks
QB = 512  # q block in attention inner loop
NQB = S // QB  # 4
GC = 8  # 256-col projection groups per (w, pair)
NIT = 2 * NQB * SC  # 128 global attention iterations


def build():
    nc = bacc.Bacc(
        "TRN2",
        target_bir_lowering=False,
        debug=False,
        enable_asserts=False,
        num_devices=NCORES,
    )
    hidb = nc.dram_tensor("hidb", [S, H], BF16, kind="ExternalInput").ap()
    # weights pre-arranged on host to [128, 3, JC, DPC] (k,q,v stacked) so
    # the load is ONE DMA of 128 contiguous 12KB lines
    wkqv = nc.dram_tensor("wkqv", [128, 3, JC, DPC], BF16, kind="ExternalInput").ap()
    # mask | bqs | bks packed into one [128, SC+4] f32 tensor
    consts = nc.dram_tensor("consts", [128, SC + 4], F32, kind="ExternalInput").ap()
    bvs = nc.dram_tensor("bvs", [1, DPC], BF16, kind="ExternalInput").ap()
    out = nc.dram_tensor("out", [S, DPC], F32, kind="ExternalOutput").ap()

    with tile.TileContext(nc) as tc:
        with (
            tc.tile_pool(name="persist", bufs=1) as persist,
            tc.tile_pool(name="etp", bufs=8) as etp,
            tc.tile_pool(name="ctsp", bufs=2) as ctsp,
            tc.tile_pool(name="rcp", bufs=4) as rcp,
            tc.tile_pool(name="scps", bufs=3, space="PSUM") as scps,
            tc.tile_pool(name="ctxps", bufs=1, space="PSUM") as ctxps,
        ):
            # dummy operands for the PE warmup chain, first so the DVE
            # memsets clear quickly
            dummyw = persist.tile([128, 128], BF16, tag="dummyw")
            nc.vector.memset(dummyw[:], 0.0)
            dummy = persist.tile([128, 512], BF16, tag="dummy")
            nc.vector.memset(dummy[:], 0.0)

            # All startup DMAs on the sync queue in an explicit chain order:
            # every DMA around the X-bar transposes serializes on the
            # previous one's completion anyway (X-bar quiescence), so the
            # emission order IS the transfer order and every separate DMA op
            # adds its issue+transfer latency to the chain. Hence one merged
            # consts DMA and one merged weights DMA before the transposes.
            consts_sb = persist.tile([128, SC + 4], F32, tag="consts")
            nc.sync.dma_start(consts_sb[:], consts)
            mask_sb = consts_sb[:, 0:SC]
            bqs_sb = consts_sb[:, SC : SC + 2]
            bks_sb = consts_sb[:, SC + 2 : SC + 4]
            wt = persist.tile([128, 3, JC, DPC], BF16, tag="wkqv", name="w_kqv")
            nc.sync.dma_start(wt[:], wkqv)
            w_sb = {"wk": wt[:, 0], "wq": wt[:, 1], "wv": wt[:, 2]}
            bvs_sb = persist.tile([1, DPC], BF16, tag="bvs")
            nc.sync.dma_start(bvs_sb[:], bvs)

            hidA = [
                persist.tile([128, 512], BF16, tag=f"hA{j}", name=f"hA{j}")
                for j in range(JC)
            ]
            hidB = [
                persist.tile([128, 512], BF16, tag=f"hB{j}", name=f"hB{j}")
                for j in range(JC)
            ]
            hidC = [
                persist.tile([128, 1024], BF16, tag=f"hC{j}", name=f"hC{j}")
                for j in range(JC)
            ]
            for j in range(JC):
                nc.sync.dma_start_transpose(
                    out=hidA[j][:], in_=hidb[0:512, j * 128 : (j + 1) * 128]
                )
            for j in range(JC):
                nc.sync.dma_start_transpose(
                    out=hidB[j][:], in_=hidb[512:1024, j * 128 : (j + 1) * 128]
                )
            for j in range(JC):
                nc.sync.dma_start_transpose(
                    out=hidC[j][:], in_=hidb[1024:S, j * 128 : (j + 1) * 128]
                )

            ones1_f = persist.tile([1, 128], F32, tag="ones1f")
            nc.vector.memset(ones1_f[:], 1.0)
            # warm the ACT exp table during startup (after the DMA issues)
            warm = persist.tile([1, 1], F32, tag="warm")
            nc.scalar.activation(warm[:], ones1_f[:, 0:1], EXP)

            ident = persist.tile([128, 128], F32, tag="ident")
            make_identity(nc, ident[:])
            ident_bf = persist.tile([128, 128], BF16, tag="identbf")
            nc.vector.tensor_copy(ident_bf[:], ident[:])
            ones1 = persist.tile([1, 128], BF16, tag="ones1")
            nc.vector.tensor_copy(ones1[:], ones1_f[:])
            ones4_f = persist.tile([128, HPC], F32, tag="ones4f")
            nc.vector.memset(ones4_f[:], 1.0)

            # persistent activations
            qT = [
                persist.tile([128, S], BF16, tag=f"qT{p}", name=f"qT{p}")
                for p in range(2)
            ]
            kT = [
                persist.tile([128, S], BF16, tag=f"kT{p}", name=f"kT{p}")
                for p in range(2)
            ]
            v_sb = [
                persist.tile([128, HPC, 65], BF16, tag=f"v{s}", name=f"v{s}")
                for s in range(SC)
            ]
            out_sb = [
                persist.tile([128, DPC], F32, tag=f"o{s}", name=f"o{s}")
                for s in range(SC)
            ]

            def hid_g(g, j):
                # 256-wide column group g of hiddenT chunk j (g = s//256)
                if g < 2:
                    return hidA[j][:, (g % 2) * 256 : (g % 2 + 1) * 256]
                if g < 4:
                    return hidB[j][:, (g % 2) * 256 : (g % 2 + 1) * 256]
                return hidC[j][:, (g - 4) * 256 : (g - 3) * 256]

            def hid_s(s, j):
                # 128-wide column chunk s of hiddenT chunk j
                if s < 4:
                    return hidA[j][:, s * 128 : (s + 1) * 128]
                if s < 8:
                    return hidB[j][:, (s - 4) * 128 : (s - 3) * 128]
                return hidC[j][:, (s - 8) * 128 : (s - 7) * 128]

            # PE warmup: ramp the p-state (full clock needs 3us continuous
            # busy) while the DMA chain delivers weights + first transposes.
            # Results are discarded.
            wps = scps.tile([128, 512], F32, tag="sc", name="warmup_ps")
            for _ in range(30):
                nc.tensor.matmul(wps[:], dummyw[:], dummy[:], start=True, stop=True)

            def qk_group(wname, dst, bias, p, g):
                # one 256-wide output group of the qT/kT projection
                ps = scps.tile([128, 256], F32, tag="sc", name="projps_t")
                for j in range(JC):
                    nc.tensor.matmul(
                        ps[:],
                        w_sb[wname][:, j, p * 128 : (p + 1) * 128],
                        hid_g(g, j),
                        start=(j == 0),
                        stop=(j == JC - 1),
                    )
                nc.vector.tensor_scalar_add(
                    dst[p][:, g * 256 : (g + 1) * 256], ps[:], bias[:, p : p + 1]
                )

            def v_proj(s):
                ps = scps.tile([128, DPC], F32, tag="sc", name="vps_t")
                for j in range(JC):
                    nc.tensor.matmul(
                        ps[:],
                        hid_s(s, j),
                        w_sb["wv"][:, j, :],
                        start=(j == 0),
                        stop=False,
                    )
                nc.tensor.matmul(ps[:], ones1[:], bvs_sb[:], start=False, stop=True)
                ps3 = ps.rearrange("p (h c) -> p h c", h=HPC)
                nc.vector.tensor_copy(v_sb[s][:, :, 0:HD], ps3[:])
                nc.vector.tensor_copy(
                    v_sb[s][:, :, HD : HD + 1],
                    ones4_f[:].rearrange("p (h o) -> p h o", o=1),
                )

            # critical-path projections before the attention stream starts:
            # k cols 0-255 (covers sc k=0,1) and q cols 0-511 (covers qb 0)
            qk_group("wk", kT, bks_sb, 0, 0)
            qk_group("wq", qT, bqs_sb, 0, 0)
            qk_group("wq", qT, bqs_sb, 0, 1)

            # Filler schedules. dlfill runs BEFORE the sc(i+2) emission (only
            # the kg groups whose output sc(i+2) reads -- anything emitted
            # before a sc inflates the PE-counter threshold the matching exp
            # waits on, so keep this set minimal). fillers runs AFTER the
            # sc(i+2) emission and before ctx(i):
            #   qg/kg before the sc emission of their first consumer
            #   v_proj(s) at iter <= s (consumed by ctx of pair0 qb0 iter s)
            #   within an iter: early-data (hidA/hidB) groups FIRST so they
            #   are not queued behind a filler stalled on late transposes
            dlfill = {}
            fillers = {}

            def kg(p, g):
                return lambda: qk_group("wk", kT, bks_sb, p, g)

            def qg(p, g):
                return lambda: qk_group("wq", qT, bqs_sb, p, g)

            def vp(s):
                return lambda s=s: v_proj(s)

            for it, g in ((0, 1), (2, 2), (6, 4), (8, 5), (10, 6), (12, 7)):
                dlfill[it] = [kg(0, g)]
            # A = s0-511, B = s512-1023, C = s1024-2047 availability
            for it, fns in {
                0: [vp(0)],
                1: [vp(1)],
                2: [vp(2)],
                3: [vp(3), kg(0, 3)],
                4: [kg(1, 0), vp(4)],
                5: [kg(1, 1), vp(5)],
                6: [qg(0, 2), vp(6)],
                7: [qg(0, 3), vp(7)],
                8: [kg(1, 2), vp(8)],
                9: [kg(1, 3), vp(9)],
                10: [qg(1, 0), vp(10)],
                11: [qg(1, 1), vp(11)],
                12: [qg(1, 2), vp(12)],
                13: [qg(1, 3), vp(13)],
                14: [vp(14)],
                15: [vp(15)],
                # cruise-phase fillers (all C-dep, data long since ready)
                17: [qg(0, 4)],
                18: [qg(0, 5)],
                20: [kg(1, 4)],
                24: [kg(1, 5)],
                33: [qg(0, 6)],
                34: [qg(0, 7)],
                36: [kg(1, 6)],
                40: [kg(1, 7)],
                82: [qg(1, 4)],
                86: [qg(1, 5)],
                98: [qg(1, 6)],
                102: [qg(1, 7)],
            }.items():
                fillers[it] = fns

            def it_pqk(i):
                return i // 64, (i % 64) // 16, i % 16

            sts = {}

            def emit_scores(i):
                pair, qb, k = it_pqk(i)
                st = scps.tile([128, 2 * QB], F32, tag="sc", name="sc_t")
                qs = qb * QB
                # adjacent emission, opposite row groups -> the PE runs
                # these two K=64 matmuls concurrently
                nc.tensor.matmul(
                    st[:, 0:QB],
                    kT[pair][0:64, k * 128 : (k + 1) * 128],
                    qT[pair][0:64, qs : qs + QB],
                    start=True,
                    stop=True,
                )
                nc.tensor.matmul(
                    st[:, QB : 2 * QB],
                    kT[pair][64:128, k * 128 : (k + 1) * 128],
                    qT[pair][64:128, qs : qs + QB],
                    start=True,
                    stop=True,
                )
                sts[i] = st

            emit_scores(0)
            emit_scores(1)
            ctxs = {}
            for i in range(NIT):
                pair, qb, k = it_pqk(i)
                h0, h1 = 2 * pair, 2 * pair + 1
                if k == 0:
                    ctxs[0] = ctxps.tile([65, QB], F32, tag="ctx0", name="ctx0")
                    ctxs[1] = ctxps.tile([65, QB], F32, tag="ctx1", name="ctx1")
                st = sts.pop(i)
                et = etp.tile([128, 2 * QB], BF16, tag="et", name="et_t")
                nc.scalar.activation(
                    et[:], st[:], EXP, bias=mask_sb[:, k : k + 1], scale=1.0
                )
                for fn in dlfill.get(i, ()):
                    fn()
                if i + 2 < NIT:
                    emit_scores(i + 2)
                for fn in fillers.get(i, ()):
                    fn()
                nc.tensor.matmul(
                    ctxs[0][:],
                    v_sb[k][:, h0, :],
                    et[:, 0:QB],
                    start=(k == 0),
                    stop=(k == SC - 1),
                )
                nc.tensor.matmul(
                    ctxs[1][:],
                    v_sb[k][:, h1, :],
                    et[:, QB : 2 * QB],
                    start=(k == 0),
                    stop=(k == SC - 1),
                )
                if k == SC - 1:
                    # finalize this qb block: copy ctx to SBUF bf16, PE-
                    # transpose 128-col chunks (PSUM reuses the two ctx
                    # banks, alternating for pipeline depth 2), normalize
                    # by the denominator column, stage to out_sb
                    ctss = {}
                    for hh, ctx in ((0, ctxs[0]), (1, ctxs[1])):
                        cts = ctsp.tile([65, QB], BF16, tag="cts", name="cts_t")
                        nc.vector.tensor_copy(cts[:], ctx[:])
                        ctss[hh] = cts
                    # on the very last qb there is no next sc, so borrow the
                    # idle score-PSUM slots for extra transpose pipelining
                    # during the drain
                    if i == NIT - 1:
                        slots = [
                            (ctxps, "ctx0"),
                            (ctxps, "ctx1"),
                            (scps, "sc"),
                            (scps, "sc"),
                        ]
                    else:
                        slots = [(ctxps, "ctx0"), (ctxps, "ctx1")]
                    nt = 0
                    for ci in range(QB // 128):
                        for hh in (0, 1):
                            h = 2 * pair + hh
                            pool, tag = slots[nt % len(slots)]
                            tp = pool.tile([128, 65], BF16, tag=tag, name="tp_t")
                            nt += 1
                            nc.tensor.transpose(
                                tp[:],
                                ctss[hh][:, ci * 128 : (ci + 1) * 128],
                                ident_bf[0:65, 0:65],
                            )
                            rc = rcp.tile([128, 1], F32, tag="rc", name="rc_t")
                            nc.vector.reciprocal(rc[:], tp[:, HD : HD + 1])
                            qc = qb * (QB // 128) + ci
                            nc.vector.tensor_scalar_mul(
                                out_sb[qc][:, h * HD : (h + 1) * HD],
                                tp[:, 0:HD],
                                rc[:],
                            )
                    # stream this (pair, qb) half-block of the output out
                    for ci in range(QB // 128):
                        qc = qb * (QB // 128) + ci
                        nc.sync.dma_start(
                            out[
                                qc * 128 : (qc + 1) * 128,
                                pair * 128 : (pair + 1) * 128,
                            ],
                            out_sb[qc][:, pair * 128 : (pair + 1) * 128],
                        )

    nc.compile()
    return nc


def make_in_maps(hidden_states, attention_mask, Wq, bq, Wk, bk, Wv, bv):
    hidden_states = np.asarray(hidden_states, dtype=np.float32)
    attention_mask = np.asarray(attention_mask, dtype=np.float32)
    Wq = np.asarray(Wq, dtype=np.float32)
    bq = np.asarray(bq, dtype=np.float32)
    Wk = np.asarray(Wk, dtype=np.float32)
    bk = np.asarray(bk, dtype=np.float32)
    Wv = np.asarray(Wv, dtype=np.float32)
    bv = np.asarray(bv, dtype=np.float32)
    bf = ml_dtypes.bfloat16

    def warr(w):
        # [H, DPC] -> [128, JC, DPC]: partition-major layout so the device
        # DMA is contiguous per partition
        return w.reshape(JC, 128, DPC).transpose(1, 0, 2)

    in_maps = []
    for c in range(NCORES):
        b = c // 4
        g = c % 4
        rows = slice(g * DPC, (g + 1) * DPC)
        wkqv = np.stack(
            [
                warr(Wk[rows, :].T),
                warr((Wq[rows, :] * 0.125).T),
                warr(Wv[rows, :].T),
            ],
            axis=1,
        )
        consts = np.concatenate(
            [
                attention_mask[b, 0, 0, :].reshape(SC, 128).T,
                (bq[rows] * 0.125).reshape(2, 128).T,
                bk[rows].reshape(2, 128).T,
            ],
            axis=1,
        )
        in_maps.append(
            {
                "hidb": np.ascontiguousarray(hidden_states[b]).astype(bf),
                "wkqv": np.ascontiguousarray(wkqv).astype(bf),
                "consts": np.ascontiguousarray(consts.astype(np.float32)),
                "bvs": np.ascontiguousarray(bv[rows].reshape(1, DPC)).astype(bf),
            }
        )
    return in_maps


def gather(results):
    full = np.empty((B, S, H), dtype=np.float32)
    for c in range(NCORES):
        b = c // 4
        g = c % 4
        full[b, :, g * DPC : (g + 1) * DPC] = results[c]["out"]
    return full


_NC = None


def kernel(hidden_states, attention_mask, Wq, bq, Wk, bk, Wv, bv, **run_kwargs):
    global _NC
    if _NC is None:
        _NC = build()
    in_maps = make_in_maps(hidden_states, attention_mask, Wq, bq, Wk, bk, Wv, bv)
    res = run_bass_kernel_spmd(_NC, in_maps, core_ids=list(range(NCORES)), **run_kwargs)
    out = gather(res.results)
    if run_kwargs:
        kernel.last_result = res
    return out
